# revision 12
# baseline (speedup 1.0000x reference)
"""CrossCompressUnit TRN2 kernel.

v_out = v * (e.w_vv) + e * (v.w_ev) + bias_v
e_out = v * (e.w_ve) + e * (v.w_ee) + bias_e

Data-parallel over batch across 8 NeuronCores (2048 rows/core).
Host stacks ve = [e, v] so each supertile moves with ONE 2MB DMA each way.

Per 128-row block ("granule"):
  - ONE fused VectorE tensor_tensor over a 4D access pattern computes all
    four products (e*w_vv, v*w_ev, e*w_ve, v*w_ee) in a single pass
  - per-row dots via 4 ScalarE activation accum_out reduces (sink in PSUM)
  - v_out: diagonal matmuls on TensorE accumulating in PSUM
    (diag = identity * per-partition dot via VectorE tensor_scalar),
    evacuated by ScalarE
  - e_out: t3 = v*s_ve on ScalarE (activation scale), t4 = e*s_ee on
    VectorE tensor_scalar, sum on VectorE tensor_tensor -> SBUF directly
"""

import numpy as np
from contextlib import ExitStack

import concourse.bass as bass
import concourse.bacc as bacc
import concourse.tile as tile
from concourse import mybir
from concourse import bass_utils

NCORES = 8
B = 16384
D = 1024
RPC = B // NCORES          # rows per core
P = 128                    # partitions
NBLK = RPC // P            # 16 row-blocks per core
NPG = 2                    # row-blocks per supertile (2MB stacked DMAs)
NST = NBLK // NPG          # supertiles per core

F32 = mybir.dt.float32

_built = {}
LAST_RESULT = None
TRACE = False


def _build(with_bias: bool):
    nc = bacc.Bacc(
        "TRN2",
        target_bir_lowering=False,
        debug=False,
        enable_asserts=False,
        num_devices=NCORES,
    )

    # host interleaves per row-block: ve[n, 0] = e rows, ve[n, 1] = v rows
    ve_d = nc.dram_tensor("ve", [NBLK, 2, P, D], F32, kind="ExternalInput").ap()
    w_d = nc.dram_tensor("wcat", [4 * D], F32, kind="ExternalInput").ap()
    id_d = nc.dram_tensor("ident", [P, P], F32, kind="ExternalInput").ap()
    if with_bias:
        b_d = nc.dram_tensor("bcat", [1, 2 * D], F32, kind="ExternalInput").ap()
    # out[n, 0] = v_out rows, out[n, 1] = e_out rows
    o_d = nc.dram_tensor("veout", [NBLK, 2, P, D], F32, kind="ExternalOutput").ap()

    # [128, 16, 2, 1024]: partition = row within block, n = row-block, s = e/v
    ver = ve_d.rearrange("n s p d -> p n s d")
    our = o_d.rearrange("n s p d -> p n s d")

    MULT = mybir.AluOpType.mult
    ADD = mybir.AluOpType.add
    COPY = mybir.ActivationFunctionType.Copy

    with tile.TileContext(nc) as tc:
        with ExitStack() as ctx:
            singles = ctx.enter_context(tc.tile_pool(name="singles", bufs=1))
            io_pool = ctx.enter_context(tc.tile_pool(name="io", bufs=3))
            m_pool = ctx.enter_context(tc.tile_pool(name="m", bufs=2))
            t_pool = ctx.enter_context(tc.tile_pool(name="t", bufs=2))
            dg_pool = ctx.enter_context(tc.tile_pool(name="diag", bufs=3))
            sm_pool = ctx.enter_context(tc.tile_pool(name="small", bufs=4))
            ps_pool = ctx.enter_context(
                tc.tile_pool(name="psum", bufs=2, space="PSUM")
            )

            # weights broadcast across partitions: [128, 4, 1024]
            # order: w_vv, w_ev, w_ve, w_ee multiplied against (e, v, e, v)
            wb = singles.tile([P, 4, D], F32)
            w_bcast = bass.AP(
                tensor=w_d.tensor, offset=w_d.offset, ap=[[0, P], w_d.ap[0]]
            )
            nc.gpsimd.dma_start(out=wb, in_=w_bcast)

            ident = singles.tile([P, P], F32)
            nc.sync.dma_start(out=ident, in_=id_d)

            # sink for the ACT-reduce primary outputs
            garbage = singles.tile([P, D], F32)

            if with_bias:
                ones1 = singles.tile([1, P], F32)
                nc.vector.memset(ones1, 1.0)
                brow = singles.tile([1, 2 * D], F32)
                nc.sync.dma_start(out=brow, in_=b_d)
                beb = singles.tile([P, D], F32)
                b_bcast = bass.AP(
                    tensor=b_d.tensor,
                    offset=b_d.offset + D,
                    ap=[[0, P], [1, D]],
                )
                nc.gpsimd.dma_start(out=beb, in_=b_bcast)

            for t in range(NST):
                blk = slice(t * NPG, (t + 1) * NPG)
                ve = io_pool.tile([P, NPG, 2, D], F32, tag="ve")
                nc.sync.dma_start(out=ve, in_=ver[:, blk, :, :])
                ou = io_pool.tile([P, NPG, 2, D], F32, tag="ou")

                for g in range(NPG):
                    eg = ve[:, g, 0, :]
                    vg = ve[:, g, 1, :]

                    # fused products: (e, v, e, v) * (w_vv, w_ev, w_ve, w_ee)
                    # in0 4D AP walks e-block, v-block, e-block, v-block
                    base = ve[:, g, :, :]
                    in0 = bass.AP(
                        tensor=base.tensor,
                        offset=base.offset,
                        ap=[base.ap[0], [0, 2], base.ap[1], base.ap[2]],
                    )
                    m4 = m_pool.tile([P, 4, D], F32, tag="m4")
                    nc.vector.tensor_tensor(out=m4, in0=in0, in1=wb, op=MULT)

                    # dots: s0 = e.w_vv, s1 = v.w_ev, s2 = e.w_ve, s3 = v.w_ee
                    s = sm_pool.tile([P, 4], F32, tag="dots")
                    for k in range(4):
                        nc.scalar.activation(
                            out=garbage,
                            in_=m4[:, k, :],
                            func=COPY,
                            accum_out=s[:, k : k + 1],
                        )

                    # v_out = s0*v + s1*e via PE diag matmuls
                    dgs = dg_pool.tile([P, 2, P], F32, tag="dg")
                    nc.vector.tensor_scalar_mul(dgs[:, 0, :], ident, s[:, 0:1])
                    nc.vector.tensor_scalar_mul(dgs[:, 1, :], ident, s[:, 1:2])

                    vps = ps_pool.tile([P, D], F32, tag="vps")
                    H = 512
                    for h in range(D // H):
                        sl = slice(h * H, (h + 1) * H)
                        nc.tensor.matmul(
                            vps[:, sl], dgs[:, 0, :], vg[:, sl],
                            start=True, stop=False,
                        )
                        nc.tensor.matmul(
                            vps[:, sl], dgs[:, 1, :], eg[:, sl],
                            start=False, stop=not with_bias,
                        )
                        if with_bias:
                            nc.tensor.matmul(
                                vps[:, sl], ones1, brow[0:1, sl],
                                start=False, stop=True,
                            )
                    nc.scalar.copy(out=ou[:, g, 0, :], in_=vps)

                    # e_out = s2*v + s3*e on ScalarE/VectorE directly in SBUF
                    t3 = t_pool.tile([P, D], F32, tag="t3")
                    nc.scalar.activation(
                        out=t3, in_=vg, func=COPY, scale=s[:, 2:3]
                    )
                    t4 = t_pool.tile([P, D], F32, tag="t4")
                    nc.vector.tensor_scalar_mul(t4, eg, s[:, 3:4])
                    nc.vector.tensor_tensor(
                        out=ou[:, g, 1, :], in0=t3, in1=t4, op=ADD
                    )
                    if with_bias:
                        nc.vector.tensor_tensor(
                            out=ou[:, g, 1, :],
                            in0=ou[:, g, 1, :],
                            in1=beb,
                            op=ADD,
                        )

                nc.sync.dma_start(out=our[:, blk, :, :], in_=ou)

    nc.compile()
    return nc


def _get(with_bias: bool):
    if with_bias not in _built:
        _built[with_bias] = _build(with_bias)
    return _built[with_bias]


def kernel(v, e, weight_vv, weight_ev, weight_ve, weight_ee, bias_v, bias_e):
    global LAST_RESULT
    v = np.asarray(v, dtype=np.float32)
    e = np.asarray(e, dtype=np.float32)
    bias_v = np.asarray(bias_v, dtype=np.float32)
    bias_e = np.asarray(bias_e, dtype=np.float32)
    with_bias = bool(np.any(bias_v) or np.any(bias_e))

    nc = _get(with_bias)

    wcat = np.concatenate(
        [
            np.asarray(w, dtype=np.float32).reshape(-1)
            for w in (weight_vv, weight_ev, weight_ve, weight_ee)
        ]
    )
    ident = np.eye(P, dtype=np.float32)
    bcat = np.concatenate([bias_v.reshape(-1), bias_e.reshape(-1)]).reshape(1, -1)

    # interleave per 128-row block: [NBLK_total, 2, P, D], s=0 e, s=1 v
    ve = np.ascontiguousarray(
        np.stack([e.reshape(-1, P, D), v.reshape(-1, P, D)], axis=1)
    )

    in_maps = []
    for c in range(NCORES):
        blocks = slice(c * NBLK, (c + 1) * NBLK)
        m = {"ve": ve[blocks], "wcat": wcat, "ident": ident}
        if with_bias:
            m["bcat"] = bcat
        in_maps.append(m)

    res = bass_utils.run_bass_kernel_spmd(
        nc, in_maps, core_ids=list(range(NCORES)), trace=TRACE
    )
    LAST_RESULT = res

    out = np.concatenate([r["veout"] for r in res.results], axis=0)  # [NBLK*8,2,P,D]
    vout = out[:, 0].reshape(B, D)
    eout = out[:, 1].reshape(B, D)
    return (vout, eout)


# revision 15
# speedup vs baseline: 1.2068x; 1.2068x over previous
"""CrossCompressUnit TRN2 kernel.

v_out = v * (e.w_vv) + e * (v.w_ev) + bias_v
e_out = v * (e.w_ve) + e * (v.w_ee) + bias_e

Data-parallel over batch across 8 NeuronCores (2048 rows/core).
Host interleaves e/v per 128-row block so each supertile moves with ONE
2MB DMA each way on the Sync HWDGE ring (one-time loads ride the Scalar
HWDGE ring so they never delay the input stream).

Per 128-row block ("granule"):
  - four per-row dot products, each ONE fused VectorE scalar_tensor_tensor:
    out=(src*1.0)*w_k with accum_out = the dot
  - v_out: diagonal matmuls on TensorE accumulating in PSUM
    (diag = identity scaled per-partition on ScalarE), evacuated by ScalarE
  - e_out: t4 = e*s_ee on ScalarE (activation scale), then ONE fused
    VectorE scalar_tensor_tensor: (v*s_ve) + t4 -> SBUF directly
"""

import numpy as np
from contextlib import ExitStack

import concourse.bass as bass
import concourse.bacc as bacc
import concourse.tile as tile
from concourse import mybir
from concourse import bass_utils

NCORES = 8
B = 16384
D = 1024
RPC = B // NCORES          # rows per core
P = 128                    # partitions
NBLK = RPC // P            # 16 row-blocks per core
NPG = 2                    # row-blocks per supertile (2MB stacked DMAs)
NST = NBLK // NPG          # supertiles per core

F32 = mybir.dt.float32

_built = {}
LAST_RESULT = None
TRACE = False


def _build(with_bias: bool):
    nc = bacc.Bacc(
        "TRN2",
        target_bir_lowering=False,
        debug=False,
        enable_asserts=False,
        num_devices=NCORES,
    )

    # host interleaves per row-block: ve[n, 0] = e rows, ve[n, 1] = v rows
    ve_d = nc.dram_tensor("ve", [NBLK, 2, P, D], F32, kind="ExternalInput").ap()
    w_d = nc.dram_tensor("wcat", [P, 4 * D], F32, kind="ExternalInput").ap()
    id_d = nc.dram_tensor("ident", [P, P], F32, kind="ExternalInput").ap()
    if with_bias:
        b_d = nc.dram_tensor("bcat", [1, 2 * D], F32, kind="ExternalInput").ap()
    # out[n, 0] = v_out rows, out[n, 1] = e_out rows
    o_d = nc.dram_tensor("veout", [NBLK, 2, P, D], F32, kind="ExternalOutput").ap()

    # [128, 16, 2, 1024]: partition = row within block, n = row-block, s = e/v
    ver = ve_d.rearrange("n s p d -> p n s d")
    our = o_d.rearrange("n s p d -> p n s d")

    MULT = mybir.AluOpType.mult
    ADD = mybir.AluOpType.add
    COPY = mybir.ActivationFunctionType.Copy

    with tile.TileContext(nc) as tc:
        with ExitStack() as ctx:
            singles = ctx.enter_context(tc.tile_pool(name="singles", bufs=1))
            io_pool = ctx.enter_context(tc.tile_pool(name="io", bufs=3))
            t_pool = ctx.enter_context(tc.tile_pool(name="t", bufs=3))
            dg_pool = ctx.enter_context(tc.tile_pool(name="diag", bufs=3))
            sm_pool = ctx.enter_context(tc.tile_pool(name="small", bufs=6))
            ps_pool = ctx.enter_context(
                tc.tile_pool(name="psum", bufs=2, space="PSUM")
            )

            # one-time loads on the Scalar HWDGE ring (keeps Sync ring
            # exclusively for the ve/out stream)
            # weights pre-broadcast on host: [128, 4, 1024]
            # order: w_vv, w_ev, w_ve, w_ee multiplied against (e, v, e, v)
            wb = singles.tile([P, 4, D], F32)
            nc.scalar.dma_start(out=wb, in_=w_d)

            ident = singles.tile([P, P], F32)
            nc.scalar.dma_start(out=ident, in_=id_d)

            # sink for the dot-STT primary outputs
            garbage = singles.tile([P, D], F32)

            if with_bias:
                ones1 = singles.tile([1, P], F32)
                nc.vector.memset(ones1, 1.0)
                brow = singles.tile([1, 2 * D], F32)
                nc.scalar.dma_start(out=brow, in_=b_d)
                beb = singles.tile([P, D], F32)
                b_bcast = bass.AP(
                    tensor=b_d.tensor,
                    offset=b_d.offset + D,
                    ap=[[0, P], [1, D]],
                )
                nc.gpsimd.dma_start(out=beb, in_=b_bcast)

            for t in range(NST):
                blk = slice(t * NPG, (t + 1) * NPG)
                ve = io_pool.tile([P, NPG, 2, D], F32, tag="ve")
                nc.sync.dma_start(out=ve, in_=ver[:, blk, :, :])
                ou = io_pool.tile([P, NPG, 2, D], F32, tag="ou")

                for g in range(NPG):
                    eg = ve[:, g, 0, :]
                    vg = ve[:, g, 1, :]

                    # dots: s0 = e.w_vv, s1 = v.w_ev, s2 = e.w_ve, s3 = v.w_ee
                    # each is ONE fused DVE op: out=(src*1)*w_k, accum=dot
                    s = sm_pool.tile([P, 4], F32, tag="dots")
                    for k, src in enumerate((eg, vg, eg, vg)):
                        nc.vector.scalar_tensor_tensor(
                            out=garbage,
                            in0=src,
                            scalar=1.0,
                            in1=wb[:, k, :],
                            op0=MULT,
                            op1=MULT,
                            accum_out=s[:, k : k + 1],
                        )

                    # v_out = s0*v + s1*e via PE diag matmuls
                    dgs = dg_pool.tile([P, 2, P], F32, tag="dg")
                    nc.scalar.activation(
                        out=dgs[:, 0, :], in_=ident, func=COPY, scale=s[:, 0:1]
                    )
                    nc.scalar.activation(
                        out=dgs[:, 1, :], in_=ident, func=COPY, scale=s[:, 1:2]
                    )

                    vps = ps_pool.tile([P, D], F32, tag="vps")
                    H = 512
                    for h in range(D // H):
                        sl = slice(h * H, (h + 1) * H)
                        nc.tensor.matmul(
                            vps[:, sl], dgs[:, 0, :], vg[:, sl],
                            start=True, stop=False,
                        )
                        nc.tensor.matmul(
                            vps[:, sl], dgs[:, 1, :], eg[:, sl],
                            start=False, stop=not with_bias,
                        )
                        if with_bias:
                            nc.tensor.matmul(
                                vps[:, sl], ones1, brow[0:1, sl],
                                start=False, stop=True,
                            )
                    nc.scalar.copy(out=ou[:, g, 0, :], in_=vps)

                    # e_out = s2*v + s3*e: t4 on ScalarE, fused mix-add on DVE
                    t4 = t_pool.tile([P, D], F32, tag="t4")
                    nc.scalar.activation(
                        out=t4, in_=eg, func=COPY, scale=s[:, 3:4]
                    )
                    nc.vector.scalar_tensor_tensor(
                        out=ou[:, g, 1, :],
                        in0=vg,
                        scalar=s[:, 2:3],
                        in1=t4,
                        op0=MULT,
                        op1=ADD,
                    )
                    if with_bias:
                        nc.vector.tensor_tensor(
                            out=ou[:, g, 1, :],
                            in0=ou[:, g, 1, :],
                            in1=beb,
                            op=ADD,
                        )

                nc.sync.dma_start(out=our[:, blk, :, :], in_=ou)

    nc.compile()
    return nc


def _get(with_bias: bool):
    if with_bias not in _built:
        _built[with_bias] = _build(with_bias)
    return _built[with_bias]


def kernel(v, e, weight_vv, weight_ev, weight_ve, weight_ee, bias_v, bias_e):
    global LAST_RESULT
    v = np.asarray(v, dtype=np.float32)
    e = np.asarray(e, dtype=np.float32)
    bias_v = np.asarray(bias_v, dtype=np.float32)
    bias_e = np.asarray(bias_e, dtype=np.float32)
    with_bias = bool(np.any(bias_v) or np.any(bias_e))

    nc = _get(with_bias)

    wcat = np.concatenate(
        [
            np.asarray(w, dtype=np.float32).reshape(-1)
            for w in (weight_vv, weight_ev, weight_ve, weight_ee)
        ]
    )
    wcat = np.ascontiguousarray(np.broadcast_to(wcat, (P, 4 * D)))
    ident = np.eye(P, dtype=np.float32)
    bcat = np.concatenate([bias_v.reshape(-1), bias_e.reshape(-1)]).reshape(1, -1)

    # interleave per 128-row block: [NBLK_total, 2, P, D], s=0 e, s=1 v
    ve = np.ascontiguousarray(
        np.stack([e.reshape(-1, P, D), v.reshape(-1, P, D)], axis=1)
    )

    in_maps = []
    for c in range(NCORES):
        blocks = slice(c * NBLK, (c + 1) * NBLK)
        m = {"ve": ve[blocks], "wcat": wcat, "ident": ident}
        if with_bias:
            m["bcat"] = bcat
        in_maps.append(m)

    res = bass_utils.run_bass_kernel_spmd(
        nc, in_maps, core_ids=list(range(NCORES)), trace=TRACE
    )
    LAST_RESULT = res

    out = np.concatenate([r["veout"] for r in res.results], axis=0)  # [NBLK*8,2,P,D]
    vout = out[:, 0].reshape(B, D)
    eout = out[:, 1].reshape(B, D)
    return (vout, eout)


# revision 16
# speedup vs baseline: 1.2625x; 1.0461x over previous
"""CrossCompressUnit TRN2 kernel.

v_out = v * (e.w_vv) + e * (v.w_ev) + bias_v
e_out = v * (e.w_ve) + e * (v.w_ee) + bias_e

Data-parallel over batch across 8 NeuronCores (2048 rows/core).
Host interleaves e/v per 128-row block so each supertile moves with ONE
2MB DMA each way on the Sync HWDGE ring (one-time loads ride the Scalar
HWDGE ring so they never delay the input stream).

Per 128-row block ("granule"):
  - four per-row dot products, each ONE fused VectorE scalar_tensor_tensor:
    out=(src*1.0)*w_k with accum_out = the dot
  - v_out: diagonal matmuls on TensorE accumulating in PSUM
    (diag = identity scaled per-partition on ScalarE), evacuated by ScalarE
  - e_out: t4 = e*s_ee on ScalarE (activation scale), then ONE fused
    VectorE scalar_tensor_tensor: (v*s_ve) + t4 -> SBUF directly
"""

import numpy as np
from contextlib import ExitStack

import concourse.bass as bass
import concourse.bacc as bacc
import concourse.tile as tile
from concourse import mybir
from concourse import bass_utils

NCORES = 8
B = 16384
D = 1024
RPC = B // NCORES          # rows per core
P = 128                    # partitions
NBLK = RPC // P            # 16 row-blocks per core
NPG = 2                    # row-blocks per supertile (2MB stacked DMAs)
NST = NBLK // NPG          # supertiles per core

F32 = mybir.dt.float32

_built = {}
LAST_RESULT = None
TRACE = False


def _build(with_bias: bool):
    nc = bacc.Bacc(
        "TRN2",
        target_bir_lowering=False,
        debug=False,
        enable_asserts=False,
        num_devices=NCORES,
    )

    # host interleaves per row-block: ve[n, 0] = e rows, ve[n, 1] = v rows
    ve_d = nc.dram_tensor("ve", [NBLK, 2, P, D], F32, kind="ExternalInput").ap()
    w_d = nc.dram_tensor("wcat", [P, 4 * D], F32, kind="ExternalInput").ap()
    id_d = nc.dram_tensor("ident", [P, P], F32, kind="ExternalInput").ap()
    if with_bias:
        b_d = nc.dram_tensor("bcat", [1, 2 * D], F32, kind="ExternalInput").ap()
    # out[n, 0] = v_out rows, out[n, 1] = e_out rows
    o_d = nc.dram_tensor("veout", [NBLK, 2, P, D], F32, kind="ExternalOutput").ap()

    # [128, 16, 2, 1024]: partition = row within block, n = row-block, s = e/v
    ver = ve_d.rearrange("n s p d -> p n s d")
    our = o_d.rearrange("n s p d -> p n s d")

    MULT = mybir.AluOpType.mult
    ADD = mybir.AluOpType.add
    COPY = mybir.ActivationFunctionType.Copy

    with tile.TileContext(nc) as tc:
        with ExitStack() as ctx:
            singles = ctx.enter_context(tc.tile_pool(name="singles", bufs=1))
            io_pool = ctx.enter_context(tc.tile_pool(name="io", bufs=4))
            t_pool = ctx.enter_context(tc.tile_pool(name="t", bufs=3))
            dg_pool = ctx.enter_context(tc.tile_pool(name="diag", bufs=3))
            sm_pool = ctx.enter_context(tc.tile_pool(name="small", bufs=6))
            ps_pool = ctx.enter_context(
                tc.tile_pool(name="psum", bufs=2, space="PSUM")
            )

            # one-time loads on the Scalar HWDGE ring (keeps Sync ring
            # exclusively for the ve/out stream)
            # weights pre-broadcast on host: [128, 4, 1024]
            # order: w_vv, w_ev, w_ve, w_ee multiplied against (e, v, e, v)
            wbs = []
            for k in range(4):
                wbk = singles.tile([P, D], F32, name=f"wb{k}")
                nc.scalar.dma_start(out=wbk, in_=w_d[:, k * D : (k + 1) * D])
                wbs.append(wbk)

            ident = singles.tile([P, P], F32)
            nc.gpsimd.dma_start(out=ident, in_=id_d)

            if with_bias:
                ones1 = singles.tile([1, P], F32)
                nc.vector.memset(ones1, 1.0)
                brow = singles.tile([1, 2 * D], F32)
                nc.scalar.dma_start(out=brow, in_=b_d)
                beb = singles.tile([P, D], F32)
                b_bcast = bass.AP(
                    tensor=b_d.tensor,
                    offset=b_d.offset + D,
                    ap=[[0, P], [1, D]],
                )
                nc.gpsimd.dma_start(out=beb, in_=b_bcast)

            for t in range(NST):
                blk = slice(t * NPG, (t + 1) * NPG)
                ve = io_pool.tile([P, NPG, 2, D], F32, tag="ve")
                nc.sync.dma_start(out=ve, in_=ver[:, blk, :, :])
                ou = io_pool.tile([P, NPG, 2, D], F32, tag="ou")

                for g in range(NPG):
                    eg = ve[:, g, 0, :]
                    vg = ve[:, g, 1, :]

                    # dots: s0 = e.w_vv, s1 = v.w_ev, s2 = e.w_ve, s3 = v.w_ee
                    # each is ONE fused DVE op: out=(src*1)*w_k, accum=dot
                    s = sm_pool.tile([P, 4], F32, tag="dots")
                    garbage = t_pool.tile([P, D], F32, tag="garbage")
                    for k, src in enumerate((eg, vg, eg, vg)):
                        nc.vector.scalar_tensor_tensor(
                            out=garbage,
                            in0=src,
                            scalar=1.0,
                            in1=wbs[k],
                            op0=MULT,
                            op1=MULT,
                            accum_out=s[:, k : k + 1],
                        )

                    # v_out = s0*v + s1*e via PE diag matmuls
                    dgs = dg_pool.tile([P, 2, P], F32, tag="dg")
                    nc.scalar.activation(
                        out=dgs[:, 0, :], in_=ident, func=COPY, scale=s[:, 0:1]
                    )
                    nc.scalar.activation(
                        out=dgs[:, 1, :], in_=ident, func=COPY, scale=s[:, 1:2]
                    )

                    vps = ps_pool.tile([P, D], F32, tag="vps")
                    H = 512
                    for h in range(D // H):
                        sl = slice(h * H, (h + 1) * H)
                        nc.tensor.matmul(
                            vps[:, sl], dgs[:, 0, :], vg[:, sl],
                            start=True, stop=False,
                        )
                        nc.tensor.matmul(
                            vps[:, sl], dgs[:, 1, :], eg[:, sl],
                            start=False, stop=not with_bias,
                        )
                        if with_bias:
                            nc.tensor.matmul(
                                vps[:, sl], ones1, brow[0:1, sl],
                                start=False, stop=True,
                            )
                    nc.scalar.copy(out=ou[:, g, 0, :], in_=vps)

                    # e_out = s2*v + s3*e: t4 on ScalarE, fused mix-add on DVE
                    t4 = t_pool.tile([P, D], F32, tag="t4")
                    nc.scalar.activation(
                        out=t4, in_=eg, func=COPY, scale=s[:, 3:4]
                    )
                    nc.vector.scalar_tensor_tensor(
                        out=ou[:, g, 1, :],
                        in0=vg,
                        scalar=s[:, 2:3],
                        in1=t4,
                        op0=MULT,
                        op1=ADD,
                    )
                    if with_bias:
                        nc.vector.tensor_tensor(
                            out=ou[:, g, 1, :],
                            in0=ou[:, g, 1, :],
                            in1=beb,
                            op=ADD,
                        )

                nc.sync.dma_start(out=our[:, blk, :, :], in_=ou)

    nc.compile()
    return nc


def _get(with_bias: bool):
    if with_bias not in _built:
        _built[with_bias] = _build(with_bias)
    return _built[with_bias]


def kernel(v, e, weight_vv, weight_ev, weight_ve, weight_ee, bias_v, bias_e):
    global LAST_RESULT
    v = np.asarray(v, dtype=np.float32)
    e = np.asarray(e, dtype=np.float32)
    bias_v = np.asarray(bias_v, dtype=np.float32)
    bias_e = np.asarray(bias_e, dtype=np.float32)
    with_bias = bool(np.any(bias_v) or np.any(bias_e))

    nc = _get(with_bias)

    wcat = np.concatenate(
        [
            np.asarray(w, dtype=np.float32).reshape(-1)
            for w in (weight_vv, weight_ev, weight_ve, weight_ee)
        ]
    )
    wcat = np.ascontiguousarray(np.broadcast_to(wcat, (P, 4 * D)))
    ident = np.eye(P, dtype=np.float32)
    bcat = np.concatenate([bias_v.reshape(-1), bias_e.reshape(-1)]).reshape(1, -1)

    # interleave per 128-row block: [NBLK_total, 2, P, D], s=0 e, s=1 v
    ve = np.ascontiguousarray(
        np.stack([e.reshape(-1, P, D), v.reshape(-1, P, D)], axis=1)
    )

    in_maps = []
    for c in range(NCORES):
        blocks = slice(c * NBLK, (c + 1) * NBLK)
        m = {"ve": ve[blocks], "wcat": wcat, "ident": ident}
        if with_bias:
            m["bcat"] = bcat
        in_maps.append(m)

    res = bass_utils.run_bass_kernel_spmd(
        nc, in_maps, core_ids=list(range(NCORES)), trace=TRACE
    )
    LAST_RESULT = res

    out = np.concatenate([r["veout"] for r in res.results], axis=0)  # [NBLK*8,2,P,D]
    vout = out[:, 0].reshape(B, D)
    eout = out[:, 1].reshape(B, D)
    return (vout, eout)
